# revision 1
# baseline (speedup 1.0000x reference)
"""Trainium2 Bass kernel for grouped-attention MoE routing.

Math (derived from the nn.Module):
  gate  = softmax(mlp(maxpool(conv(x))) + mlp(avgpool(conv(x))))      (B,45)
  sel   = sorted(top22(mean_b gate))                                  (22,)
  Per expert e with u = x[:, sel[e], :]:
    energy[l,m] = (a_e*u_l + g_e) * u_m   (rank-1; scalars a,g from weights)
    attn = softmax_m(energy);  s_l = sum_m u_m attn[l,m]
    y_l  = P_e*s_l + Q_e;      A[:,sel[e],:] = y * gate[:,sel[e]]
  G = x * A (flat);  return (G, A_flat)

Strategy: pure data parallel over batch on 8 cores; two launches with the
45-float routing reduction mediated on host (equivalent of the all-reduce).
"""

import numpy as np
from contextlib import ExitStack

import bass_rust
import concourse.bass as bass
import concourse.mybir as mybir
import concourse.tile as tile
from concourse.bass_utils import run_bass_kernel_spmd

_MULTIWAIT_OK = ("InstNoOp", "InstAllEngineBarrier",
                 "InstEventSemaphore", "InstUnconditionalBranch")


def legalize_sync_waits(nc):
    """walrus codegen on this stack rejects >1 sync wait on most
    instructions; hoist extra waits onto same-engine NoOps."""
    for func in nc.m.functions:
        for block in func.blocks:
            il = block.instructions
            out = []
            for inst in il:
                tname = type(inst).__name__
                si = getattr(inst, "sync_info", None)
                waits = list(si.on_wait) if si is not None else []
                if tname not in _MULTIWAIT_OK and len(waits) > 1:
                    for k, w in enumerate(waits):
                        nop = mybir.InstNoOp(
                            name=f"{inst.name}-synop{k}", ins=[], outs=[])
                        nop.engine = inst.engine
                        nop.sync_info = bass_rust.SyncInfo(
                            on_wait=[w], on_update=[])
                        out.append(nop)
                    inst.sync_info = bass_rust.SyncInfo(
                        on_wait=[], on_update=list(inst.sync_info.on_update))
                out.append(inst)
            il.clear()
            il.extend(out)

B, C, L, E = 8192, 45, 21, 22
NCORES = 8
BC = B // NCORES          # rows per core
P = 128                   # SBUF partitions
NT = BC // P              # batch tiles per core
CL = C * L                # 945
F32 = mybir.dt.float32
BF16 = mybir.dt.bfloat16
AF = mybir.ActivationFunctionType
ALU = mybir.AluOpType
AX = mybir.AxisListType

# channel groups for the gating conv matmul: 8 groups of <=6 channels
GROUPS = [list(range(g, min(g + 6, C))) for g in range(0, C, 6)]
BIG_BUFS = 2


def _ap(base, extra_free):
    """Custom free-dim access pattern on an SBUF tile slice.

    base: AP from tile[:, a:b]; extra_free: list of [step,count] replacing
    the free dims (partition dim kept)."""
    return bass.AP(tensor=base.tensor, offset=base.offset,
                   ap=[base.ap[0]] + extra_free)


def build_gate_program(repeat=1):
    nc = bass.Bass()
    x = nc.declare_dram_parameter("x", [BC, CL], F32, isOutput=False)
    # per-group block-diag gc_w^T (rows: (i,l) pairs), bias rows separate
    wblk = nc.declare_dram_parameter("wblk", [126, len(GROUPS) * 126], F32,
                                     isOutput=False)
    wbias = nc.declare_dram_parameter("wbias", [1, len(GROUPS) * 126], F32,
                                      isOutput=False)
    w1mx = nc.declare_dram_parameter("w1mx", [C, 25], F32, isOutput=False)
    w1av = nc.declare_dram_parameter("w1av", [C, 25], F32, isOutput=False)
    b1r = nc.declare_dram_parameter("b1r", [1, 25], F32, isOutput=False)
    w2 = nc.declare_dram_parameter("w2", [25, C], F32, isOutput=False)
    b2r = nc.declare_dram_parameter("b2r", [1, C], F32, isOutput=False)
    ident = nc.declare_dram_parameter("ident", [P, P], F32, isOutput=False)
    gate_o = nc.declare_dram_parameter("gate", [BC, C], F32, isOutput=True)
    gsum_o = nc.declare_dram_parameter("gsum", [C, 1], F32, isOutput=True)

    with tile.TileContext(nc) as tc, ExitStack() as ctx:
        singles = ctx.enter_context(tc.tile_pool(name="singles", bufs=1))
        xs = ctx.enter_context(tc.tile_pool(name="xs", bufs=2))
        work = ctx.enter_context(tc.tile_pool(name="work", bufs=3))
        small = ctx.enter_context(tc.tile_pool(name="small", bufs=4))
        ps = ctx.enter_context(tc.tile_pool(name="ps", bufs=2, space="PSUM"))
        psm = ctx.enter_context(tc.tile_pool(name="psm", bufs=1, space="PSUM"))
        pst = ctx.enter_context(tc.tile_pool(name="pst", bufs=1, space="PSUM"))
        pss = ctx.enter_context(tc.tile_pool(name="pss", bufs=1, space="PSUM"))

        # All PE-read tensors funnel through DVE so every matmul needs at
        # most one sync wait (fp32 self-loading matmul ISA limit).
        def dve_const(dram, p, n):
            raw = singles.tile([p, n], F32, name="raw_" + dram.name)
            nc.sync.dma_start(out=raw, in_=dram[:, :])
            t = singles.tile([p, n], F32, name="sb_" + dram.name)
            nc.vector.tensor_copy(out=t, in_=raw)
            return t

        sb_id = dve_const(ident, P, P)
        sb_wblk = dve_const(wblk, 126, len(GROUPS) * 126)
        sb_wbias = dve_const(wbias, 1, len(GROUPS) * 126)
        sb_w1mx = dve_const(w1mx, C, 25)
        sb_w1av = dve_const(w1av, C, 25)
        sb_b1r = dve_const(b1r, 1, 25)
        sb_w2 = dve_const(w2, 25, C)
        sb_b2r = dve_const(b2r, 1, C)
        ones_col = singles.tile([P, 1], F32)
        nc.vector.memset(ones_col, 1.0)
        ones_row = singles.tile([1, P], F32)
        nc.vector.memset(ones_row, 1.0)
        # dummy PE op: advances PE's observed DVE clock past the consts
        warm_ps = pss.tile([1, P], F32)
        nc.tensor.transpose(warm_ps, ones_col, sb_id)

        gsum_ps = pss.tile([C, 1], F32)

        def mlp_branch(h_sb, w1_sb):
            """h_sb (P,45) -> tanh((tanh(h@w1+b1))@w2+b2) as (P,45) SBUF."""
            hT_ps = psm.tile([C, P], F32, tag="mlpT")
            nc.tensor.transpose(hT_ps, h_sb, sb_id)
            hT = work.tile([C, P], F32, tag="hT_sb")
            nc.vector.tensor_copy(out=hT, in_=hT_ps)
            p1 = psm.tile([P, 25], F32, tag="mlpP")
            nc.tensor.matmul(p1, hT, w1_sb, start=True, stop=False)
            nc.tensor.matmul(p1, ones_row, sb_b1r, start=False, stop=True)
            p1c = small.tile([P, 25], F32, tag="p1c")
            nc.vector.tensor_copy(out=p1c, in_=p1)
            t1 = small.tile([P, 25], F32, tag="t1")
            nc.scalar.activation(out=t1, in_=p1c, func=AF.Tanh)
            t1d = small.tile([P, 25], F32, tag="t1d")
            nc.vector.tensor_copy(out=t1d, in_=t1)
            t1T_ps = psm.tile([25, P], F32, tag="mlpT")
            nc.tensor.transpose(t1T_ps, t1d, sb_id)
            t1T = work.tile([25, P], F32, tag="t1T_sb")
            nc.vector.tensor_copy(out=t1T, in_=t1T_ps)
            p2 = psm.tile([P, C], F32, tag="mlpP")
            nc.tensor.matmul(p2, t1T, sb_w2, start=True, stop=False)
            nc.tensor.matmul(p2, ones_row, sb_b2r, start=False, stop=True)
            p2c = small.tile([P, C], F32, tag="p2c")
            nc.vector.tensor_copy(out=p2c, in_=p2)
            z = small.tile([P, C], F32, tag="z")
            nc.scalar.activation(out=z, in_=p2c, func=AF.Tanh)
            return z

        for t in range(NT):
            xt = xs.tile([P, CL], F32)
            nc.sync.dma_start(out=xt, in_=x[t * P:(t + 1) * P, :])

            for _r in range(repeat):
                # conv: per channel-group transpose + block-diag matmul
                temp_ps = [pst.tile([P, 504], F32, tag="tempA", name="tempA"),
                           pst.tile([P, 504], F32, tag="tempB", name="tempB")]
                for g, chans in enumerate(GROUPS):
                    w = len(chans) * L  # 126 or 63
                    xT_ps = ps.tile([126, P], F32, tag="xT")
                    nc.tensor.transpose(xT_ps[0:w, :],
                                        xt[:, chans[0] * L:chans[0] * L + w],
                                        sb_id)
                    lhs = work.tile([126, P], F32, tag="lhs")
                    nc.vector.tensor_copy(out=lhs[0:w, :], in_=xT_ps[0:w, :])
                    half, slot = divmod(g, 4)
                    dst = temp_ps[half][:, slot * 126:(slot + 1) * 126]
                    nc.tensor.matmul(dst, lhs[0:w, :],
                                     sb_wblk[0:w, g * 126:(g + 1) * 126],
                                     start=True, stop=False)
                    nc.tensor.matmul(dst, ones_row,
                                     sb_wbias[:, g * 126:(g + 1) * 126],
                                     start=False, stop=True)

                # mx/av pools over the 21 conv output channels
                mx = small.tile([P, 48], F32, tag="mx")
                av = small.tile([P, 48], F32, tag="av")
                for half in range(2):
                    src = _ap(temp_ps[half][:, 0:504], [[126, 4], [21, 6], [1, L]])
                    nc.vector.tensor_reduce(out=mx[:, half * 24:half * 24 + 24],
                                            in_=src, axis=AX.X, op=ALU.max)
                    nc.vector.tensor_reduce(out=av[:, half * 24:half * 24 + 24],
                                            in_=src, axis=AX.X, op=ALU.add)

                zmx = mlp_branch(mx[:, 0:C], sb_w1mx)
                zav = mlp_branch(av[:, 0:C], sb_w1av)
                z = small.tile([P, C], F32, tag="zsum")
                nc.vector.tensor_add(out=z, in0=zmx, in1=zav)

                # softmax over the 45 channels
                m1 = small.tile([P, 1], F32, tag="m1")
                nc.vector.tensor_reduce(out=m1, in_=z, axis=AX.X, op=ALU.max)
                nm = small.tile([P, 1], F32, tag="nm")
                nc.vector.tensor_scalar_mul(out=nm, in0=m1, scalar1=-1.0)
                eg = small.tile([P, C], F32, tag="eg")
                ssum = small.tile([P, 1], F32, tag="ssum")
                nc.scalar.activation(out=eg, in_=z, func=AF.Exp, bias=nm,
                                     accum_out=ssum)
                rs = small.tile([P, 1], F32, tag="rs")
                nc.vector.reciprocal(out=rs, in_=ssum)
                gt = small.tile([P, C], F32, tag="gt")
                nc.vector.tensor_scalar_mul(out=gt, in0=eg, scalar1=rs)
                nc.sync.dma_start(out=gate_o[t * P:(t + 1) * P, :], in_=gt)

                nc.tensor.matmul(gsum_ps, gt, ones_col,
                                 start=(t == 0), stop=(t == NT - 1))


        gs_sb = singles.tile([C, 1], F32)
        nc.vector.tensor_copy(out=gs_sb, in_=gsum_ps)
        nc.sync.dma_start(out=gsum_o[:, :], in_=gs_sb)
    legalize_sync_waits(nc)
    return nc


def build_attn_program(sel, repeat=1, pool_experts=0):
    """sel: sorted list of 22 selected channels (python ints, baked in).
    repeat>1 re-runs the compute body (same I/O) for timing isolation.
    pool_experts: how many experts' big elementwise muls run on GpSimd
    (pool) instead of DVE, to parallelize the two engines."""
    # runs of consecutive channels -> contiguous slices in both x and expert idx
    runs = []  # (chan0, e0, len)
    i = 0
    while i < E:
        j = i
        while j + 1 < E and sel[j + 1] == sel[j] + 1:
            j += 1
        runs.append((sel[i], i, j - i + 1))
        i = j + 1

    EL = E * L            # 462
    ELM = E * L * L       # 9702

    nc = bass.Bass()
    x = nc.declare_dram_parameter("x", [BC, CL], F32, isOutput=False)
    gsel = nc.declare_dram_parameter("gsel", [BC, E], F32, isOutput=False)
    avec = nc.declare_dram_parameter("avec", [EL], F32, isOutput=False)
    gvec = nc.declare_dram_parameter("gvec", [EL], F32, isOutput=False)
    pvec = nc.declare_dram_parameter("pvec", [E], F32, isOutput=False)
    qvec = nc.declare_dram_parameter("qvec", [E], F32, isOutput=False)
    a_o = nc.declare_dram_parameter("asel", [BC, EL], F32, isOutput=True)
    g_o = nc.declare_dram_parameter("gout", [BC, EL], F32, isOutput=True)

    with tile.TileContext(nc) as tc, ExitStack() as ctx:
        singles = ctx.enter_context(tc.tile_pool(name="singles", bufs=1))
        xs = ctx.enter_context(tc.tile_pool(name="xs", bufs=2))
        big = ctx.enter_context(tc.tile_pool(name="big", bufs=BIG_BUFS))
        mid = ctx.enter_context(tc.tile_pool(name="mid", bufs=2))
        outs = ctx.enter_context(tc.tile_pool(name="outs", bufs=2))

        def bconst(dram, n):
            base = dram[:]
            t = singles.tile([P, n], F32, name="bc_" + dram.name)
            nc.gpsimd.dma_start(
                out=t, in_=bass.AP(tensor=base.tensor, offset=base.offset,
                                   ap=[[0, P], [1, n]]))
            return t

        aB = bconst(avec, EL)
        gB = bconst(gvec, EL)
        pB = bconst(pvec, E)
        qB = bconst(qvec, E)

        for t in range(NT):
            xt = xs.tile([P, CL], F32)
            nc.sync.dma_start(out=xt, in_=x[t * P:(t + 1) * P, :])
            gs = xs.tile([P, E], F32, tag="gs")
            nc.sync.dma_start(out=gs, in_=gsel[t * P:(t + 1) * P, :])

            for _r in range(repeat):
                # gather the 22 selected channels once; all later ops contiguous
                u = mid.tile([P, EL], F32, tag="u")
                for (c0, e0, n) in runs:
                    nc.vector.tensor_copy(out=u[:, e0 * L:(e0 + n) * L],
                                          in_=xt[:, c0 * L:(c0 + n) * L])

                # kappa[b,(e,l)] = a_e * u[b,e,l] + g_e
                kap = mid.tile([P, EL], F32, tag="kap")
                nc.vector.tensor_mul(out=kap, in0=u, in1=aB)
                nc.gpsimd.tensor_add(out=kap, in0=kap, in1=gB)

                # energy[b,(e,l,m)] = kappa[b,e,l] * u[b,e,m]; exp in place
                en = big.tile([P, ELM], F32, tag="en")
                ed = E - pool_experts  # experts on DVE

                def en_split(op_dve, op_pool):
                    for eng_mul, e0, ne in ((op_dve, 0, ed),
                                            (op_pool, ed, E - ed)):
                        if ne == 0:
                            continue
                        o = _ap(en[:, e0 * L * L:(e0 + ne) * L * L],
                                [[L * L, ne], [L, L], [1, L]])
                        kl = _ap(kap[:, e0 * L:(e0 + ne) * L],
                                 [[L, ne], [1, L], [0, L]])
                        um = _ap(u[:, e0 * L:(e0 + ne) * L],
                                 [[L, ne], [0, L], [1, L]])
                        eng_mul(o, kl, um)

                def mul_en_kap(o, kl, um):
                    nc.vector.tensor_mul(out=o, in0=kl, in1=um)

                def mul_en_kap_pool(o, kl, um):
                    nc.gpsimd.tensor_mul(out=o, in0=kl, in1=um)

                en_split(mul_en_kap, mul_en_kap_pool)
                nc.scalar.activation(out=en, in_=en, func=AF.Exp)

                den = mid.tile([P, EL], F32, tag="den")
                nc.vector.tensor_reduce(
                    out=den, in_=_ap(en[:, 0:ELM], [[L, EL], [1, L]]),
                    axis=AX.X, op=ALU.add)

                # en <- en * u_m (numerator weights), then reduce
                def mul_num(e0, ne):
                    o = _ap(en[:, e0 * L * L:(e0 + ne) * L * L],
                            [[L * L, ne], [L, L], [1, L]])
                    um = _ap(u[:, e0 * L:(e0 + ne) * L],
                             [[L, ne], [0, L], [1, L]])
                    return o, um

                o, um = mul_num(0, ed)
                nc.vector.tensor_mul(out=o, in0=o, in1=um)
                if E - ed:
                    o, um = mul_num(ed, E - ed)
                    nc.gpsimd.tensor_mul(out=o, in0=o, in1=um)
                num = mid.tile([P, EL], F32, tag="num")
                nc.vector.tensor_reduce(
                    out=num, in_=_ap(en[:, 0:ELM], [[L, EL], [1, L]]),
                    axis=AX.X, op=ALU.add)

                nc.vector.reciprocal(out=den, in_=den)
                nc.gpsimd.tensor_mul(out=num, in0=num, in1=den)  # s

                # A = s * (gate*P)_rep + (gate*Q)_rep ; G = A * u
                gp = mid.tile([P, E], F32, tag="gp")
                nc.gpsimd.tensor_mul(out=gp, in0=gs, in1=pB)
                gq = mid.tile([P, E], F32, tag="gq")
                nc.gpsimd.tensor_mul(out=gq, in0=gs, in1=qB)
                at = outs.tile([P, EL], F32, tag="at")
                nc.gpsimd.tensor_mul(out=_ap(at[:, 0:EL], [[L, E], [1, L]]),
                                     in0=_ap(num[:, 0:EL], [[L, E], [1, L]]),
                                     in1=_ap(gp[:, 0:E], [[1, E], [0, L]]))
                nc.gpsimd.tensor_add(out=_ap(at[:, 0:EL], [[L, E], [1, L]]),
                                     in0=_ap(at[:, 0:EL], [[L, E], [1, L]]),
                                     in1=_ap(gq[:, 0:E], [[1, E], [0, L]]))
                gt = outs.tile([P, EL], F32, tag="gt")
                nc.vector.tensor_mul(out=gt, in0=at, in1=u)
                nc.sync.dma_start(out=a_o[t * P:(t + 1) * P, :], in_=at)
                nc.sync.dma_start(out=g_o[t * P:(t + 1) * P, :], in_=gt)
    legalize_sync_waits(nc)
    return nc


def _host_params(inputs):
    gc_w, gc_b = inputs["gc_w"], inputs["gc_b"]
    ng = len(GROUPS)
    wblk = np.zeros((126, ng * 126), np.float32)
    wbias = np.zeros((1, ng * 126), np.float32)
    for g, chans in enumerate(GROUPS):
        for k, _ in enumerate(chans):
            c0 = g * 126 + k * L
            wblk[k * L:(k + 1) * L, c0:c0 + L] = gc_w.T
            wbias[0, c0:c0 + L] = gc_b
    w1mx = inputs["w1"].T.astype(np.float32)
    w1av = (inputs["w1"].T / L).astype(np.float32)
    b1r = inputs["b1"][None, :].astype(np.float32)
    w2 = inputs["w2"].T.astype(np.float32)
    b2r = inputs["b2"][None, :].astype(np.float32)
    return wblk, wbias, w1mx, w1av, b1r, w2, b2r


_CACHE = {}


def kernel(**inputs):
    inputs = {k: np.ascontiguousarray(np.asarray(v)) for k, v in inputs.items()}
    x = inputs["x"].astype(np.float32).reshape(B, CL)
    wblk, wbias, w1mx, w1av, b1r, w2, b2r = _host_params(inputs)
    ident = np.eye(P, dtype=np.float32)
    cores = list(range(NCORES))

    if "gate" not in _CACHE:
        _CACHE["gate"] = build_gate_program()
    nc1 = _CACHE["gate"]
    maps1 = [{"x": x[i * BC:(i + 1) * BC], "wblk": wblk, "wbias": wbias,
              "w1mx": w1mx, "w1av": w1av, "b1r": b1r, "w2": w2, "b2r": b2r,
              "ident": ident} for i in cores]
    r1 = run_bass_kernel_spmd(nc1, maps1, cores).results
    gate = np.concatenate([r["gate"] for r in r1], 0)          # (B,45)
    mean_gate = np.sum([r["gsum"][:, 0] for r in r1], 0) / B   # (45,)
    sel = np.sort(np.argsort(-mean_gate, kind="stable")[:E])

    wq, bq = inputs["wq"], inputs["bq"]
    wk, bk = inputs["wk"], inputs["bk"]
    wv, bv = inputs["wv"], inputs["bv"]
    wo, bo = inputs["wo"], inputs["bo"]
    alpha = (wq * wk).sum(1).astype(np.float32)
    gamma = (bq * wk).sum(1).astype(np.float32)
    pv = (wo * wv).sum(1).astype(np.float32)
    qv = ((wo * bv).sum(1) + bo).astype(np.float32)
    avec = np.repeat(alpha, L)
    gvec = np.repeat(gamma, L)
    gsel = np.ascontiguousarray(gate[:, sel])

    key = tuple(sel.tolist())
    if _CACHE.get("attn_key") != key:
        _CACHE["attn"] = build_attn_program([int(s) for s in sel],
                                            pool_experts=8)
        _CACHE["attn_key"] = key
    nc2 = _CACHE["attn"]
    maps2 = [{"x": x[i * BC:(i + 1) * BC], "gsel": gsel[i * BC:(i + 1) * BC],
              "avec": avec, "gvec": gvec, "pvec": pv, "qvec": qv}
             for i in cores]
    r2 = run_bass_kernel_spmd(nc2, maps2, cores).results
    asel = np.concatenate([r["asel"] for r in r2], 0)          # (B,462)
    gout = np.concatenate([r["gout"] for r in r2], 0)

    cols = (np.repeat(sel * L, L) + np.tile(np.arange(L), E))  # (462,)
    A_full = np.zeros((B, CL), np.float32)
    G_full = np.zeros((B, CL), np.float32)
    A_full[:, cols] = asel
    G_full[:, cols] = gout
    return G_full, A_full



# revision 49
# speedup vs baseline: 3.2145x; 3.2145x over previous
"""Trainium2 Bass kernel for grouped-attention MoE routing.

Math (derived from the nn.Module):
  gate  = softmax(mlp(maxpool(conv(x))) + mlp(avgpool(conv(x))))      (B,45)
  sel   = sorted(top22(mean_b gate))                                  (22,)
  Per expert e with u = x[:, sel[e], :]:
    energy[l,m] = (a_e*u_l + g_e) * u_m   (rank-1; scalars a,g from weights)
    attn = softmax_m(energy);  s_l = sum_m u_m attn[l,m]
    y_l  = P_e*s_l + Q_e;      A[:,sel[e],:] = y * gate[:,sel[e]]
  G = x * A (flat);  return (G, A_flat)

Implementation strategy (v2):
  Launch 1 (gate): bf16 PE conv with bias folded via a 127th ones row and
    the avg-pool folded in as extra matmul columns; both MLP branches run
    through one block-diagonal matmul pair; outputs batched into one DMA.
  Launch 2 (attn): the rank-1 softmax is evaluated without the LxL energy
    tensor.  With w = exp(g_e*u) and phi = a_e*u:
      den(phi_l) = sum_m w_m exp(phi_l u_m) ~= sum_k cd_k phi_l^k W_k
      num(phi_l) = sum_m u_m w_m exp(phi_l u_m) ~= sum_k cn_k phi_l^k W_{k+1}
    where W_k = sum_m w_m u_m^k are on-device moments and cd/cn are host-
    fitted per-expert polynomial coefficients (least squares over the
    empirical tau = phi*u range; numerator fit |u|-weighted).  s = num/den.
    Per-expert degree ladder (2..6) by empirical |tau| range; experts are
    permuted so degree classes are contiguous and the nested Horner only
    touches suffix slices for the high degrees.  Everything bf16, l-major
    (l outer, e inner) so per-(b,e) coefficient broadcasts stay packed.
  Routing (45-float mean-gate reduction) is mediated on host between the
  two launches, equivalent to the all-reduce in the sharding hint.
"""

import math
import numpy as np
from contextlib import ExitStack

import bass_rust
import concourse.bass as bass
import concourse.mybir as mybir
import concourse.tile as tile
from concourse.bass_utils import run_bass_kernel_spmd

_MULTIWAIT_OK = ("InstNoOp", "InstAllEngineBarrier",
                 "InstEventSemaphore", "InstUnconditionalBranch")


def legalize_sync_waits(nc):
    """walrus codegen on this stack rejects >1 sync wait on most
    instructions; hoist extra waits onto same-engine NoOps."""
    for func in nc.m.functions:
        for block in func.blocks:
            il = block.instructions
            out = []
            for inst in il:
                tname = type(inst).__name__
                si = getattr(inst, "sync_info", None)
                waits = list(si.on_wait) if si is not None else []
                if tname not in _MULTIWAIT_OK and len(waits) > 1:
                    for k, w in enumerate(waits):
                        nop = mybir.InstNoOp(
                            name=f"{inst.name}-synop{k}", ins=[], outs=[])
                        nop.engine = inst.engine
                        nop.sync_info = bass_rust.SyncInfo(
                            on_wait=[w], on_update=[])
                        out.append(nop)
                    inst.sync_info = bass_rust.SyncInfo(
                        on_wait=[], on_update=list(inst.sync_info.on_update))
                out.append(inst)
            il.clear()
            il.extend(out)


B, C, L, E = 8192, 45, 21, 22
NCORES = 8
BC = B // NCORES          # rows per core
P = 128                   # SBUF partitions
NT = BC // P              # batch tiles per core
CL = C * L                # 945
EL = E * L                # 462
WF = NT * EL              # 3696 full-shard free width (t, l, e) l-major
F32 = mybir.dt.float32
BF16 = mybir.dt.bfloat16
AF = mybir.ActivationFunctionType
ALU = mybir.AluOpType
AX = mybir.AxisListType

NG = 8                    # conv channel groups (6,6,...,3 channels)
GCH = [list(range(g, min(g + 6, C))) for g in range(0, C, 6)]
NLEV = 8                  # moment levels W_0..W_7 held on device
DEG_MAX = 6


def _ap(base, extra_free):
    """Custom free-dim access pattern on an SBUF tile slice (partition
    dim kept from `base`)."""
    return bass.AP(tensor=base.tensor, offset=base.offset,
                   ap=[base.ap[0]] + extra_free)


def _off(base, extra_free, col_off):
    ap = bass.AP(tensor=base.tensor, offset=base.offset,
                 ap=[base.ap[0]] + extra_free)
    ap.offset = ap.offset + col_off
    return ap


# --------------------------------------------------------------------------
# Launch 1: gating network
# --------------------------------------------------------------------------

def build_gate_program():
    """Gate launch. Host uploads x TRANSPOSED and padded: XT (1024, 1024)
    with rows 0:945 = x[coreshard].T (row = (chan,l)), rows 945:1023 = 0,
    row 1023 = 1 (bias lane).  The conv+avg matmul accumulates chunk-wise
    into 3 PSUM column blocks of 15 channels x 22 outputs; no on-device
    transposes or PSUM->SBUF copies are needed for the conv at all."""
    nc = bass.Bass()
    KC = P * NT                       # 1024 padded contraction rows
    xT = nc.declare_dram_parameter("xT", [KC, KC], BF16, isOutput=False)
    # packed rhs slices for the 12 (chunk, block) matmuls: (128, 12*330)
    wpk = nc.declare_dram_parameter("wpk", [P, 12 * 330], BF16,
                                    isOutput=False)
    w1blk = nc.declare_dram_parameter("w1blk", [P, P], F32, isOutput=False)
    w2blk = nc.declare_dram_parameter("w2blk", [P, 90], F32, isOutput=False)
    ident = nc.declare_dram_parameter("ident", [P, P], F32, isOutput=False)
    gate_o = nc.declare_dram_parameter("gate", [P, NT * C], BF16,
                                       isOutput=True)

    # (chunk, block) pairs: block j covers channels 15j..15j+14 =
    # contraction rows 315j..315j+314, plus the bias lane in chunk 7
    PAIRS = [[0, 1, 2, 7], [2, 3, 4, 7], [4, 5, 6, 7]]

    with tile.TileContext(nc) as tc, ExitStack() as ctx, \
            nc.allow_low_precision(reason="bf16 gate pipeline; 2e-2 tol"):
        singles = ctx.enter_context(tc.tile_pool(name="singles", bufs=1))
        cvps = ctx.enter_context(tc.tile_pool(name="cvps", bufs=2,
                                              space="PSUM"))
        ppps = ctx.enter_context(tc.tile_pool(name="ppps", bufs=1,
                                              space="PSUM"))
        work = ctx.enter_context(tc.tile_pool(name="work", bufs=2))
        small = ctx.enter_context(tc.tile_pool(name="small", bufs=3))

        def dve_const(dram, p, n, dt=BF16):
            raw = singles.tile([p, n], dt, name="raw_" + dram.name)
            nc.sync.dma_start(out=raw, in_=dram[:, :])
            t = singles.tile([p, n], dt, name="sb_" + dram.name)
            nc.vector.tensor_copy(out=t, in_=raw)
            return t

        sb_w = dve_const(wpk, P, 12 * 330)
        sb_w1 = dve_const(w1blk, P, P, F32)
        sb_w2 = dve_const(w2blk, P, 90, F32)
        sb_id = dve_const(ident, P, P, F32)

        # xT loads: per-tile DMA of the (128, 8x128) lhsT panel so tile 0
        # can start early.  lhs[p, c*128+b] = XT[128c+p, 128t+b]
        xb = xT[:, :]
        lhsT = []
        for t in range(NT):
            lt = singles.tile([P, NT * P], BF16, name=f"lhsT{t}")
            ap = bass.AP(tensor=xb.tensor, offset=xb.offset,
                         ap=[[KC, P], [P * KC, NT], [1, P]])
            ap.offset = ap.offset + t * P
            nc.sync.dma_start(out=lt, in_=ap)
            lhsT.append(lt)

        # persistent h tiles: cols 0:90 rewritten each use; 90:128 junk is
        # zeroed once (col 95 = ones lane feeding the layer-1 bias row)
        hb = []
        for i in range(2):
            t = singles.tile([P, P], F32, name=f"hb{i}")
            nc.vector.memset(t[:, 90:P], 0.0)
            nc.vector.memset(t[:, 95:96], 1.0)
            hb.append(t)

        gate_all = singles.tile([P, NT * C], BF16)
        zall = singles.tile([P, NT * C], BF16)

        def conv_block(t):
            lt = lhsT[t]
            cvt = [cvps.tile([P, 330], F32, tag=f"cv{j}", name=f"cv{j}")
                   for j in range(3)]
            for j in range(3):
                for i, c in enumerate(PAIRS[j]):
                    nc.tensor.matmul(
                        cvt[j], lt[:, c * P:(c + 1) * P],
                        sb_w[:, (j * 4 + i) * 330:(j * 4 + i + 1) * 330],
                        start=(i == 0), stop=(i == len(PAIRS[j]) - 1))
            h = hb[t % 2]
            for j in range(3):
                nc.vector.tensor_reduce(
                    out=h[:, 15 * j:15 * j + 15],
                    in_=_ap(cvt[j][:, 0:330], [[22, 15], [1, L]]),
                    axis=AX.X, op=ALU.max)
            # avg lanes (col 21 of each 22-block): 2 on DVE, 1 on Act
            nc.vector.tensor_copy(out=h[:, 45:60],
                                  in_=_off(cvt[0], [[22, 15]], 21))
            nc.scalar.copy(out=h[:, 60:75], in_=_off(cvt[1], [[22, 15]], 21))
            nc.vector.tensor_copy(out=h[:, 75:90],
                                  in_=_off(cvt[2], [[22, 15]], 21))
            return h

        def mlp_block(t, h):
            # both MLP branches through 128x128 f32 blocks; PE transposes
            # with ones lanes via h col 95 and the saturated tanh col 127
            trm = ppps.tile([P, P], F32, tag="trm", name="trm")
            nc.tensor.transpose(trm, h, sb_id)
            hT = work.tile([P, P], F32, tag="hT", name="hT")
            nc.scalar.copy(out=hT, in_=trm)
            pp = ppps.tile([P, P], F32, tag="pp", name="pp")
            nc.tensor.matmul(pp, hT, sb_w1, start=True, stop=True)
            t1 = small.tile([P, P], F32, tag="t1", name="t1")
            nc.scalar.activation(out=t1, in_=pp, func=AF.Tanh)
            trm2 = ppps.tile([P, P], F32, tag="trm", name="trm2")
            nc.tensor.transpose(trm2, t1, sb_id)
            t1T = work.tile([P, P], F32, tag="t1T", name="t1T")
            nc.vector.tensor_copy(out=t1T, in_=trm2)
            p2 = ppps.tile([P, 90], F32, tag="pp", name="p2")
            nc.tensor.matmul(p2, t1T, sb_w2, start=True, stop=True)
            z2 = small.tile([P, 90], BF16, tag="z2", name="z2")
            nc.scalar.activation(out=z2, in_=p2, func=AF.Tanh)
            nc.vector.tensor_add(out=zall[:, t * C:(t + 1) * C],
                                 in0=z2[:, 0:45], in1=z2[:, 45:90])

        # software-pipelined emission: engines issue in program order, so
        # interleave conv(t+1) ahead of mlp(t) to let tiles overlap
        hprev = None
        for t in range(NT + 1):
            if t < NT:
                hcur = conv_block(t)
            if t >= 1:
                mlp_block(t - 1, hprev)
            hprev = hcur

        # batched softmax over all 8 tile blocks (segmented per block)
        zmax = singles.tile([P, NT], F32)
        nc.vector.tensor_reduce(out=zmax, in_=_ap(zall, [[C, NT], [1, C]]),
                                axis=AX.X, op=ALU.max)
        zmax16 = singles.tile([P, NT], BF16)
        nc.vector.tensor_copy(out=zmax16, in_=zmax)
        zsub = singles.tile([P, NT * C], BF16)
        nc.vector.tensor_sub(out=zsub, in0=zall,
                             in1=_ap(zmax16, [[1, NT], [0, C]]))
        eg = singles.tile([P, NT * C], BF16)
        nc.scalar.activation(out=eg, in_=zsub, func=AF.Exp)
        ssum = singles.tile([P, NT], F32)
        nc.vector.tensor_reduce(out=ssum, in_=_ap(eg, [[C, NT], [1, C]]),
                                axis=AX.X, op=ALU.add)
        rs = singles.tile([P, NT], BF16)
        nc.vector.reciprocal(out=rs, in_=ssum)
        nc.vector.tensor_mul(out=gate_all, in0=eg,
                             in1=_ap(rs, [[1, NT], [0, C]]))
        nc.sync.dma_start(out=gate_o[:, :], in_=gate_all)
    legalize_sync_waits(nc)
    return nc


# --------------------------------------------------------------------------
# Launch 2: expert attention via fitted moment polynomials
# --------------------------------------------------------------------------

def build_attn_program(degs):
    """degs: per-expert polynomial degree, sorted ascending (len 22)."""
    degs = list(degs)
    assert degs == sorted(degs)
    dmax = max(degs)
    # suffix start index for "experts with degree > k"
    estart = {k: next((i for i in range(E) if degs[i] > k), E)
              for k in range(dmax)}
    # first expert of each degree class (for acc initialization)
    class_start = {}
    for i, d in enumerate(degs):
        class_start.setdefault(d, i)

    nc = bass.Bass()
    u_d = nc.declare_dram_parameter("u", [BC, EL], BF16, isOutput=False)
    phi_d = nc.declare_dram_parameter("phi", [BC, EL], BF16, isOutput=False)
    gu_d = nc.declare_dram_parameter("gu", [BC, EL], BF16, isOutput=False)
    gpq_d = nc.declare_dram_parameter("gpq", [BC, 2 * E], BF16,
                                      isOutput=False)
    cd_d = nc.declare_dram_parameter("cd", [1, (NLEV - 1) * NT * E], BF16,
                                     isOutput=False)
    cn_d = nc.declare_dram_parameter("cn", [1, (NLEV - 1) * NT * E], BF16,
                                     isOutput=False)
    at_o = nc.declare_dram_parameter("at", [BC, EL], BF16, isOutput=True)
    gt_o = nc.declare_dram_parameter("gt", [BC, EL], BF16, isOutput=True)

    def shard_ap(dram, ncols):
        base = dram[:, :]
        return bass.AP(tensor=base.tensor, offset=base.offset,
                       ap=[[ncols, P], [P * ncols, NT], [1, ncols]])

    with tile.TileContext(nc) as tc, ExitStack() as ctx, \
            nc.allow_low_precision(reason="bf16 attn pipeline; 2e-2 tol"):
        sg = ctx.enter_context(tc.tile_pool(name="sg", bufs=1))

        gu = sg.tile([P, WF], BF16)
        nc.sync.dma_start(out=gu, in_=shard_ap(gu_d, EL))
        u = sg.tile([P, WF], BF16)
        nc.sync.dma_start(out=u, in_=shard_ap(u_d, EL))
        phi = sg.tile([P, WF], BF16)
        nc.sync.dma_start(out=phi, in_=shard_ap(phi_d, EL))
        gpq = sg.tile([P, NT * 2 * E], BF16)
        nc.sync.dma_start(out=gpq, in_=shard_ap(gpq_d, 2 * E))

        def bconst(dram, n, nm):
            base = dram[:, :]
            t = sg.tile([P, n], BF16, name=nm)
            nc.sync.dma_start(
                out=t, in_=bass.AP(tensor=base.tensor, offset=base.offset,
                                   ap=[[0, P], [1, n]]))
            return t

        NC_ = NT * E                      # 176 moment columns per level
        cdB = bconst(cd_d, (NLEV - 1) * NC_, "cdB")
        cnB = bconst(cn_d, (NLEV - 1) * NC_, "cnB")

        Wt = sg.tile([P, NLEV * NC_], BF16)   # moment levels W_0..W_7
        nc.vector.memset(Wt, 0.0)

        # slice helpers (l-major: free = (t, l, e); e innermost)
        def full3(tile_, e0=0, ne=E, coloff=0):
            return _off(tile_, [[EL, NT], [E, L], [1, ne]], coloff + e0)

        def wout(lev, e0=0, ne=E):
            return _off(Wt, [[E, NT], [1, ne]], lev * NC_ + e0)

        def wred_in(src, e0=0, ne=E):
            return _off(src, [[EL, NT], [1, ne], [E, L]], e0)

        def coef(ctile, lev, e0=0, ne=E):
            return _off(ctile, [[E, NT], [0, L], [1, ne]], lev * NC_ + e0)

        # w = exp(gu); moments
        w = sg.tile([P, WF], BF16)
        nc.scalar.activation(out=w, in_=gu, func=AF.Exp)
        nc.vector.tensor_reduce(out=wout(0), in_=wred_in(w), axis=AX.X,
                                op=ALU.add)
        va = sg.tile([P, WF], BF16)
        vb = sg.tile([P, WF], BF16)
        cur, nxt = va, vb
        nc.vector.tensor_mul(out=cur, in0=w, in1=u)
        nc.vector.tensor_reduce(out=wout(1), in_=wred_in(cur), axis=AX.X,
                                op=ALU.add)
        for lev in range(2, NLEV):
            # moment level `lev` is needed by experts with degree >= lev-1
            e0 = estart.get(lev - 2, E)
            ne = E - e0
            if ne <= 0:
                break
            nc.vector.tensor_mul(out=full3(nxt, e0, ne),
                                 in0=full3(cur, e0, ne),
                                 in1=full3(u, e0, ne))
            nc.vector.tensor_reduce(out=wout(lev, e0, ne),
                                    in_=wred_in(nxt, e0, ne),
                                    axis=AX.X, op=ALU.add)
            cur, nxt = nxt, cur

        # Horner coefficient tensors: D_k = W_k*cd_k, N_k = W_{k+1}*cn_k
        Dt = sg.tile([P, (NLEV - 1) * NC_], BF16)
        nc.vector.tensor_mul(out=Dt, in0=Wt[:, 0:(NLEV - 1) * NC_], in1=cdB)
        Nt = sg.tile([P, (NLEV - 1) * NC_], BF16)
        nc.vector.tensor_mul(out=Nt, in0=Wt[:, NC_:NLEV * NC_], in1=cnB)

        # nested mixed-degree Horner (experts sorted by degree ascending)
        accd = sg.tile([P, WF], BF16)
        accn = sg.tile([P, WF], BF16)
        for d, e0 in class_start.items():
            ne = (min([cs for dd, cs in class_start.items() if dd > d],
                      default=E)) - e0
            nc.vector.tensor_copy(out=full3(accd, e0, ne),
                                  in_=coef(Dt, d, e0, ne))
            nc.vector.tensor_copy(out=full3(accn, e0, ne),
                                  in_=coef(Nt, d, e0, ne))
        for k in range(dmax - 1, -1, -1):
            e0 = estart[k]
            ne = E - e0
            nc.vector.tensor_mul(out=full3(accd, e0, ne),
                                 in0=full3(accd, e0, ne),
                                 in1=full3(phi, e0, ne))
            nc.vector.tensor_add(out=full3(accd, e0, ne),
                                 in0=full3(accd, e0, ne),
                                 in1=coef(Dt, k, e0, ne))
            nc.vector.tensor_mul(out=full3(accn, e0, ne),
                                 in0=full3(accn, e0, ne),
                                 in1=full3(phi, e0, ne))
            nc.vector.tensor_add(out=full3(accn, e0, ne),
                                 in0=full3(accn, e0, ne),
                                 in1=coef(Nt, k, e0, ne))

        # s = num/den; at = s*gp + gq; gt = at*u
        # (tail ops split 16/6 experts across DVE and Pool so they overlap)
        rden = sg.tile([P, WF], BF16)
        nc.vector.reciprocal(out=rden, in_=accd)
        s = accn
        nc.vector.tensor_mul(out=s, in0=accn, in1=rden)
        at = accd
        nc.vector.tensor_mul(
            out=at, in0=s,
            in1=_ap(gpq, [[2 * E, NT], [0, L], [1, E]]))
        ESP = 16
        nc.vector.tensor_add(
            out=full3(at, 0, ESP), in0=full3(at, 0, ESP),
            in1=_off(gpq, [[2 * E, NT], [0, L], [1, ESP]], E))
        nc.gpsimd.tensor_add(
            out=full3(at, ESP, E - ESP), in0=full3(at, ESP, E - ESP),
            in1=_off(gpq, [[2 * E, NT], [0, L], [1, E - ESP]], E + ESP))
        gt = va
        nc.gpsimd.tensor_mul(out=full3(gt, ESP, E - ESP),
                             in0=full3(at, ESP, E - ESP),
                             in1=full3(u, ESP, E - ESP))
        nc.vector.tensor_mul(out=full3(gt, 0, ESP),
                             in0=full3(at, 0, ESP),
                             in1=full3(u, 0, ESP))
        nc.sync.dma_start(out=shard_ap(at_o, EL), in_=at)
        nc.sync.dma_start(out=shard_ap(gt_o, EL), in_=gt)
    legalize_sync_waits(nc)
    return nc


# --------------------------------------------------------------------------
# Host-side preparation
# --------------------------------------------------------------------------

def _gate_params(inputs):
    gc_w = inputs["gc_w"].astype(np.float64)
    gc_b = inputs["gc_b"].astype(np.float64)
    KC = P * NT
    # full conv weight: rows = (chan,l) + pad + bias lane, cols = (chan, 22)
    wfull = np.zeros((KC, 990), np.float32)
    wavvec = gc_w.mean(0)
    for i in range(C):
        wfull[i * L:(i + 1) * L, i * 22:i * 22 + L] = gc_w.T
        wfull[i * L:(i + 1) * L, i * 22 + L] = wavvec
        wfull[KC - 1, i * 22:i * 22 + L] = gc_b
        wfull[KC - 1, i * 22 + L] = gc_b.mean()
    PAIRS = [[0, 1, 2, 7], [2, 3, 4, 7], [4, 5, 6, 7]]
    wpk = np.zeros((P, 12 * 330), np.float32)
    for j in range(3):
        for i, c in enumerate(PAIRS[j]):
            wpk[:, (j * 4 + i) * 330:(j * 4 + i + 1) * 330] = \
                wfull[c * P:(c + 1) * P, j * 330:(j + 1) * 330]
    # 128x128 MLP layer-1 block: rows = transposed h cols (0:45 mx, 45:90
    # av, 95 = ones), out cols 0:50 = both branch hiddens, col 127 driven to
    # +30 via the ones row so tanh saturates to an exact 1.0 "ones" lane for
    # layer 2; all other cells zero.
    w1blk = np.zeros((128, 128), np.float32)
    w1blk[0:45, 0:25] = inputs["w1"].T
    w1blk[45:90, 25:50] = inputs["w1"].T
    w1blk[95, 0:25] = inputs["b1"]
    w1blk[95, 25:50] = inputs["b1"]
    w1blk[95, 127] = 30.0
    w2blk = np.zeros((128, 90), np.float32)
    w2blk[0:25, 0:45] = inputs["w2"].T
    w2blk[25:50, 45:90] = inputs["w2"].T
    w2blk[127, 0:45] = inputs["b2"]
    w2blk[127, 45:90] = inputs["b2"]
    return wpk, w1blk, w2blk


def _fit_exp(tsamp, K, wsamp=None, ntail=0.5):
    t = np.asarray(tsamp, np.float64)
    w = np.ones_like(t) if wsamp is None else np.asarray(wsamp, np.float64)
    tm = max(np.abs(t).max(), 1e-3)
    textra = np.linspace(-tm, tm, 64)
    t = np.concatenate([t, textra])
    w = np.concatenate([w, np.full(64, ntail * w.mean())])
    V = np.vander(t, K + 1, increasing=True) * w[:, None]
    c, *_ = np.linalg.lstsq(V, np.exp(t) * w, rcond=None)
    return c


_CACHE = {}


def kernel(**inputs):
    inputs = {k: np.ascontiguousarray(np.asarray(v)) for k, v in
              inputs.items()}
    x = inputs["x"].astype(np.float32)            # (B, C, L)
    import ml_dtypes
    bf = ml_dtypes.bfloat16

    wpk, w1blk, w2blk = _gate_params(inputs)
    cores = list(range(NCORES))
    KC = P * NT

    if "gate" not in _CACHE:
        _CACHE["gate"] = build_gate_program()
    nc1 = _CACHE["gate"]
    maps1 = []
    for i in cores:
        xt = np.zeros((KC, KC), np.float32)
        xt[0:CL, :] = x.reshape(B, CL)[i * BC:(i + 1) * BC].T
        xt[KC - 1, :] = 1.0
        maps1.append({"xT": xt.astype(bf), "wpk": wpk.astype(bf),
                      "w1blk": w1blk, "w2blk": w2blk,
                      "ident": np.eye(P, dtype=np.float32)})
    r1 = run_bass_kernel_spmd(nc1, maps1, cores).results
    # gate tiles come back as (P, NT*C): row p, block t -> batch p + t*P
    gate = np.zeros((B, C), np.float32)
    for i in cores:
        g = np.asarray(r1[i]["gate"]).astype(np.float32)
        gate[i * BC:(i + 1) * BC] = \
            g.reshape(P, NT, C).transpose(1, 0, 2).reshape(BC, C)
    mean_gate = gate.astype(np.float64).mean(0)
    sel = np.sort(np.argsort(-mean_gate, kind="stable")[:E])

    # expert scalars
    wq, bq = inputs["wq"], inputs["bq"]
    wk, bk = inputs["wk"], inputs["bk"]
    wv, bv = inputs["wv"], inputs["bv"]
    wo, bo = inputs["wo"], inputs["bo"]
    alpha = (wq * wk).sum(1).astype(np.float32)
    gamma = (bq * wk).sum(1).astype(np.float32)
    pv = (wo * wv).sum(1).astype(np.float32)
    qv = ((wo * bv).sum(1) + bo).astype(np.float32)

    usel = x[:, sel, :]                            # (B, E, L)
    # per-expert tau range -> degree ladder
    phimax = np.abs(alpha[None, :, None] * usel).max(axis=(0, 2))
    umax = np.abs(usel).max(axis=(0, 2))
    taumax = phimax * umax
    degs_raw = np.where(taumax <= 0.7, 2,
                        np.where(taumax <= 1.2, 3,
                                 np.where(taumax <= 2.0, 4, 6)))
    perm = np.argsort(degs_raw, kind="stable")     # experts by degree asc
    degs = degs_raw[perm]

    # coefficient fits per (permuted) expert
    rng = np.random.RandomState(12345)
    cd = np.zeros((NLEV - 1, E), np.float32)
    cn = np.zeros((NLEV - 1, E), np.float32)
    sub = usel[::16]                               # (B/16, E, L) samples
    for j, e in enumerate(perm):
        K = int(degs[j])
        ue = sub[:, e, :].astype(np.float64)
        tau = (alpha[e] * ue[:, :, None] * ue[:, None, :]).ravel()
        uw = np.abs(np.broadcast_to(ue[:, None, :], ue.shape[:1] + (L, L))
                    ).ravel()
        ss = rng.choice(tau.size, min(40000, tau.size), replace=False)
        cd[0:K + 1, j] = _fit_exp(tau[ss], K)
        cn[0:K + 1, j] = _fit_exp(tau[ss], K, wsamp=uw[ss] + 0.1)

    # device tensors (l-major, expert-permuted)
    uselp = usel[:, perm, :]
    u_lm = np.ascontiguousarray(uselp.transpose(0, 2, 1).reshape(B, EL))
    phi_lm = np.ascontiguousarray(
        (alpha[perm][None, :, None] * uselp).transpose(0, 2, 1)
        .reshape(B, EL))
    gu_lm = np.ascontiguousarray(
        (gamma[perm][None, :, None] * uselp).transpose(0, 2, 1)
        .reshape(B, EL))
    gsel = gate[:, sel][:, perm]
    gp = gsel * pv[perm][None, :]
    gq = gsel * qv[perm][None, :]
    gpq = np.concatenate([gp, gq], 1).astype(np.float32)   # (B, 44)

    NC_ = NT * E
    cd_full = np.tile(cd[:, None, :], (1, NT, 1)).reshape(1, (NLEV - 1) * NC_)
    cn_full = np.tile(cn[:, None, :], (1, NT, 1)).reshape(1, (NLEV - 1) * NC_)

    key = (tuple(int(d) for d in degs),)
    if _CACHE.get("attn_key") != key:
        _CACHE["attn"] = build_attn_program([int(d) for d in degs])
        _CACHE["attn_key"] = key
    nc2 = _CACHE["attn"]
    maps2 = [{"u": u_lm[i * BC:(i + 1) * BC].astype(bf),
              "phi": phi_lm[i * BC:(i + 1) * BC].astype(bf),
              "gu": gu_lm[i * BC:(i + 1) * BC].astype(bf),
              "gpq": gpq[i * BC:(i + 1) * BC].astype(bf),
              "cd": cd_full.astype(bf), "cn": cn_full.astype(bf)}
             for i in cores]
    r2 = run_bass_kernel_spmd(nc2, maps2, cores).results
    at = np.concatenate([np.asarray(r["at"]).astype(np.float32)
                         for r in r2], 0)          # (B, 462) l-major perm
    gt = np.concatenate([np.asarray(r["gt"]).astype(np.float32)
                         for r in r2], 0)

    inv = np.argsort(perm)
    at_e = at.reshape(B, L, E).transpose(0, 2, 1)[:, inv, :]   # (B,E,L)
    gt_e = gt.reshape(B, L, E).transpose(0, 2, 1)[:, inv, :]
    A_full = np.zeros((B, C, L), np.float32)
    G_full = np.zeros((B, C, L), np.float32)
    A_full[:, sel, :] = at_e
    G_full[:, sel, :] = gt_e
    return G_full.reshape(B, CL), A_full.reshape(B, CL)
